# revision 30
# baseline (speedup 1.0000x reference)
"""Multi-head attention ('general' similarity, softmax, out-proj) on 8 trn2
NeuronCores via Bass/Tile.

Sharding: core c handles batch b=c//2, query rows [qh*1024, (qh+1)*1024) with
qh=c%2. Each core computes its own projections (full K/V for its batch), all 8
heads of attention for its query slice, and its slice of the output
projection. Outputs are disjoint -> host just concatenates.

Device layout: feature-major ("transposed") so every matmul contraction runs
along SBUF partitions:
  Q_l^T[e,q]   = sum_d W_Q[d,e] Q^T[d,q]          (host supplies Q^T)
  K_hw^T[e,s]  = sum_d Wkg[d,e] K^T[d,s]          (W_gen_S folded on host)
  scores^T[k,q]= sum_e K_hw^T[e,k] Q_l^T[e,q]     (row-packed: 2 heads share PE)
  P^T          = exp(scores^T)                     (ScalarE, psum->sbuf bf16)
  headaug^T    = sum_k Vaug[k,(v,1)] P^T[k,q]     (65th 'ones' col => rowsums free)
  out[q,o]     = sum_hv (head^T/rowsum)[hv,q] W_m[hv,o]

Scheduling: one stream over (pair, kblock) positions. Per position the PE
emits both heads' score matmuls first (keeps ACT's exp stream dense), then PV
steps drain from a backlog queue to smooth PE load, then projection chunks
(paired per position so the psS ring keeps even parity — psa(p+1) must pair
with psa(p)'s exp, not psb's). Pair finalization: psum released immediately
via DVE copies, rowsum reciprocal as exp(-ln(x)) on ACT reading the rowsum
rows directly, partition-broadcast via a tiny K=33 selector matmul. The tail
folds the partial-output add into PE accumulation with an identity matmul.
All PE waits are hoisted onto NoOps so back-to-back matmuls keep pipeline
overlap.
"""
import sys
import types
from collections import deque

import numpy as np
import ml_dtypes

# ---------------------------------------------------------------- axon shim --
def _ensure_axon_hooks():
    if 'antenv.axon_hooks' in sys.modules:
        return
    try:
        from trn_agent_boot.trn_boot import _ntff_profile_via_ctypes
        hook = _ntff_profile_via_ctypes('/opt/axon/libaxon_pjrt.so')
    except Exception:
        hook = None
    mod = types.ModuleType('antenv.axon_hooks')
    mod.get_axon_ntff_profile_hook = lambda: hook
    mod.set_axon_ntff_profile_hook = lambda h: None
    sys.modules['antenv.axon_hooks'] = mod


_ensure_axon_hooks()

import concourse.bass as bass
import concourse.mybir as mybir
import concourse.tile as tile
from concourse.bass_utils import run_bass_kernel_spmd

BF16 = mybir.dt.bfloat16
F32 = mybir.dt.float32

P = 128
D = 512          # model dim (= D_K = D_V = D_OUT)
SQ = 1024        # query rows per core
SK = 2048        # key rows (full sequence)
H = 8
DH = 64
NJ = H // 2      # head pairs
NKB = SK // P    # 16 key blocks
NQB = SQ // P    # 8 query blocks
ND = D // P      # 4 feature blocks
LR = 33          # reciprocal rows tile height (rows 0 and 32 carry data)
EXPF = mybir.ActivationFunctionType.Exp
LOGF = mybir.ActivationFunctionType.Ln
MULT = mybir.AluOpType.mult


# ------------------------------------------------------- walrus workaround --
# Walrus accepts only ONE embedded sync-wait per hw instruction; additionally
# a PE instruction that carries an embedded wait loses its pipeline overlap
# with the previous matmul (~190ns refill). Hoist waits onto single-wait
# NoOps: all of them for PE instructions, all but the last elsewhere.
_SPLIT_CTR = [0]


def _split_waits(nc):
    def mk_nop(engine, wait):
        _SPLIT_CTR[0] += 1
        nop = mybir.InstNoOp(name=f"antsplitw-{_SPLIT_CTR[0]}", ins=[], outs=[])
        nop.engine = engine
        nop.sync_info = mybir.SyncInfo(on_wait=[wait], on_update=[])
        return nop

    for fn in nc.m.functions:
        for bb in fn.blocks:
            out = []
            changed = False
            for inst in bb.instructions:
                si = inst.sync_info
                waits = list(si.on_wait) if si is not None and si.on_wait else []
                keep = 0 if inst.engine == mybir.EngineType.PE else 1
                if len(waits) > keep:
                    moved = waits[:-keep] if keep else waits
                    for w in moved:
                        out.append(mk_nop(inst.engine, w))
                    si.on_wait = waits[-keep:] if keep else []
                    changed = True
                out.append(inst)
            if changed:
                bb.instructions = out


# ------------------------------------------------------------ device kernel --
def _build_nc():
    nc = bass.Bass("TRN2", target_bir_lowering=False, debug=False)

    qt_d = nc.declare_dram_parameter("qt", [D, SQ], BF16, isOutput=False)
    kt_d = nc.declare_dram_parameter("kt", [D, SK], BF16, isOutput=False)
    vt_d = nc.declare_dram_parameter("vt", [D, SK], BF16, isOutput=False)
    wq_d = nc.declare_dram_parameter("wq", [D, D], BF16, isOutput=False)
    wkg_d = nc.declare_dram_parameter("wkg", [D, D], BF16, isOutput=False)
    wv_d = nc.declare_dram_parameter("wv", [D, D], BF16, isOutput=False)
    wm_d = nc.declare_dram_parameter("wm", [D, D], BF16, isOutput=False)
    id_d = nc.declare_dram_parameter("ident", [P, P], BF16, isOutput=False)
    out_d = nc.declare_dram_parameter("out", [SQ, D], F32, isOutput=True)

    with tile.TileContext(nc) as tc:
        with tc.tile_pool(name="cst", bufs=1) as cst, \
             tc.tile_pool(name="pt", bufs=16) as ptp, \
             tc.tile_pool(name="psS", bufs=2, space="PSUM") as psS, \
             tc.tile_pool(name="psV", bufs=2, space="PSUM") as psV:

            # ---- input DMAs: two issue streams (SP + GpSimd), ordered so
            # the prologue projections' operands land just in time.
            wq = cst.tile([P, ND, D], BF16, tag="wq")
            qt = cst.tile([P, ND, SQ], BF16, tag="qt")
            kt = cst.tile([P, ND, SK], BF16, tag="kt")
            wkg = cst.tile([P, ND, D], BF16, tag="wkg")
            wv = cst.tile([P, ND, D], BF16, tag="wv")
            vt = cst.tile([P, ND, SK], BF16, tag="vt")
            wm = cst.tile([P, ND, D], BF16, tag="wm")
            ident = cst.tile([P, P], BF16, tag="ident")

            qt_r = qt_d.rearrange("(k p) q -> p k q", p=P)
            kt_r = kt_d.rearrange("(k p) s -> p k s", p=P)
            vt_r = vt_d.rearrange("(k p) s -> p k s", p=P)

            nc.sync.dma_start(wq[:], wq_d.rearrange("(k p) e -> p k e", p=P))
            nc.sync.dma_start(qt[:, :, 0:256], qt_r[:, :, 0:256])
            nc.sync.dma_start(qt[:, :, 256:512], qt_r[:, :, 256:512])
            nc.sync.dma_start(qt[:, :, 512:1024], qt_r[:, :, 512:1024])
            nc.sync.dma_start(kt[:, :, 512:1024], kt_r[:, :, 512:1024])
            nc.sync.dma_start(kt[:, :, 1024:2048], kt_r[:, :, 1024:2048])
            nc.sync.dma_start(ident[:], id_d[:])

            nc.gpsimd.dma_start(wkg[:], wkg_d.rearrange("(k p) e -> p k e", p=P))
            nc.gpsimd.dma_start(kt[:, :, 0:256], kt_r[:, :, 0:256])
            nc.gpsimd.dma_start(kt[:, :, 256:512], kt_r[:, :, 256:512])
            nc.gpsimd.dma_start(wv[:], wv_d.rearrange("(k p) e -> p k e", p=P))
            nc.gpsimd.dma_start(vt[:, :, 0:512], vt_r[:, :, 0:512])
            nc.gpsimd.dma_start(vt[:, :, 512:1024], vt_r[:, :, 512:1024])
            nc.gpsimd.dma_start(vt[:, :, 1024:2048], vt_r[:, :, 1024:2048])
            nc.gpsimd.dma_start(wm[:], wm_d.rearrange("(k p) e -> p k e", p=P))

            # ---- persistent sbuf tiles
            qlt = [cst.tile([P, SQ], BF16, tag=f"qlt{j}", name=f"qlt{j}")
                   for j in range(NJ)]
            khwt = [cst.tile([P, SK], BF16, tag=f"khwt{j}", name=f"khwt{j}")
                    for j in range(NJ)]
            vaug = [cst.tile([P, H, DH + 1], BF16, tag=f"vaug{i}", name=f"vaug{i}")
                    for i in range(NKB)]
            headt = [cst.tile([P, SQ], BF16, tag=f"headt{j}", name=f"headt{j}")
                     for j in range(NJ)]
            pout = [cst.tile([P, D], BF16, tag=f"pout{qb}", name=f"pout{qb}")
                    for qb in range(NQB)]

            # warm tile first: it gates the PE warmup matmuls
            warm = cst.tile([P, 512], BF16, tag="warm")
            nc.vector.memset(warm[:], 0.0)
            for i in range(NKB):
                nc.vector.memset(vaug[i][:, :, DH:DH + 1], 1.0)

            # selector for the rowsum-reciprocal partition broadcast:
            # row 0 -> output partitions 0:64, row 32 -> partitions 64:128
            sel2 = cst.tile([LR, P], BF16, tag="sel2")
            nc.vector.memset(sel2[:], 0.0)
            nc.vector.memset(sel2[0:1, 0:DH], 1.0)
            nc.vector.memset(sel2[32:33, DH:P], 1.0)

            # ln staging: rows 0/32 get ln(rowsum); the rest stay 0.0 so
            # exp(-x) maps them to 1.0 (never NaN).
            lg_t = [cst.tile([LR, SQ], F32, tag=f"lg{i}", name=f"lg{i}")
                    for i in range(2)]
            recr_t = [cst.tile([LR, SQ], BF16, tag=f"recr{i}", name=f"recr{i}")
                      for i in range(2)]
            for i in range(2):
                nc.vector.memset(lg_t[i][:], 0.0)
            lg3 = cst.tile([LR, SQ], F32, tag="lg3", name="lg3")
            recr3 = cst.tile([LR, SQ], BF16, tag="recr3", name="recr3")
            rbr3 = cst.tile([P, SQ], F32, tag="rbr3", name="rbr3")
            nc.vector.memset(lg3[:], 0.0)

            # ---- emission bookkeeping ------------------------------------
            mmcnt = [0.0]

            def mm(out, lhsT, rhs, w=1.0, **kw):
                nc.tensor.matmul(out, lhsT, rhs, **kw)
                mmcnt[0] += w

            # ---- projection chunks (one psS tile + one DVE copy each).
            # Chunks are scheduled in PAIRS per position (even psS parity).
            def qproj_h(j, qc):
                ps = psS.tile([P, SQ], F32, tag="psS", name="psq")
                s = qc * 512
                for k in range(ND):
                    mm(ps[:, 0:512],
                       wq[:, k, j * P:(j + 1) * P],
                       qt[:, k, s:s + 512],
                       start=(k == 0), stop=(k == ND - 1))
                nc.vector.tensor_copy(out=qlt[j][:, s:s + 512], in_=ps[:, 0:512])

            def kproj_c(j, c):
                ps = psS.tile([P, SQ], F32, tag="psS", name="psk")
                sc = c * 512
                for k in range(ND):
                    mm(ps[:, 0:512],
                       wkg[:, k, j * P:(j + 1) * P],
                       kt[:, k, sc:sc + 512],
                       start=(k == 0), stop=(k == ND - 1))
                nc.vector.tensor_copy(out=khwt[j][:, sc:sc + 512],
                                      in_=ps[:, 0:512])

            def kproj_c_sub(j, c):
                # two-psum-tile variant to keep parity when scheduled alone
                sc = c * 512
                for h in range(2):
                    ps = psS.tile([P, SQ], F32, tag="psS", name="psk2")
                    for k in range(ND):
                        mm(ps[:, 0:256],
                           wkg[:, k, j * P:(j + 1) * P],
                           kt[:, k, sc + h * 256:sc + h * 256 + 256],
                           w=0.5, start=(k == 0), stop=(k == ND - 1))
                    nc.vector.tensor_copy(
                        out=khwt[j][:, sc + h * 256:sc + h * 256 + 256],
                        in_=ps[:, 0:256])

            def qproj_h_sub(j, qc):
                s = qc * 512
                for h in range(2):
                    ps = psS.tile([P, SQ], F32, tag="psS", name="psq2")
                    for k in range(ND):
                        mm(ps[:, 0:256],
                           wq[:, k, j * P:(j + 1) * P],
                           qt[:, k, s + h * 256:s + h * 256 + 256],
                           w=0.5, start=(k == 0), stop=(k == ND - 1))
                    nc.vector.tensor_copy(
                        out=qlt[j][:, s + h * 256:s + h * 256 + 256],
                        in_=ps[:, 0:256])

            def vproj(i):
                ps = psS.tile([P, SQ], F32, tag="psS", name="psv")
                for k in range(ND):
                    mm(ps[:, 0:512],
                       vt[:, k, i * P:(i + 1) * P],
                       wv[:, k, :],
                       start=(k == 0), stop=(k == ND - 1))
                nc.vector.tensor_copy(
                    out=vaug[i][:, :, 0:DH],
                    in_=ps[:, 0:512].rearrange("p (h v) -> p h v", v=DH))

            def outproj_partial(qb):
                # pairs 0..2 of the output projection for query block qb
                ps = psS.tile([P, SQ], F32, tag="psS", name="pop")
                for j in range(NJ - 1):
                    mm(ps[:, 0:512],
                       headt[j][:, qb * P:(qb + 1) * P],
                       wm[:, j, :], start=(j == 0), stop=(j == NJ - 2))
                nc.vector.tensor_copy(out=pout[qb][:], in_=ps[:, 0:512])

            # ---- scores + exp --------------------------------------------
            pts = {}

            def scores_step(pos):
                j, t = divmod(pos, NKB)
                psa = psS.tile([P, SQ], F32, tag="psS", name="psa")
                for qc in range(2):
                    s = qc * 512
                    mm(psa[:, s:s + 512],
                       khwt[j][0:DH, t * P:(t + 1) * P],
                       qlt[j][0:DH, s:s + 512], start=True, stop=True)
                psb = psS.tile([P, SQ], F32, tag="psS", name="psb")
                for qc in range(2):
                    s = qc * 512
                    mm(psb[:, s:s + 512],
                       khwt[j][DH:P, t * P:(t + 1) * P],
                       qlt[j][DH:P, s:s + 512], start=True, stop=True,
                       tile_position=(DH, 0))
                pta = ptp.tile([P, SQ], BF16, tag="pt", name="pta")
                nc.scalar.activation(pta[:], psa[:], EXPF)
                ptb = ptp.tile([P, SQ], BF16, tag="pt", name="ptb")
                nc.scalar.activation(ptb[:], psb[:], EXPF)
                pts[pos] = (pta, ptb)

            # ---- PV ------------------------------------------------------
            pv_tiles = {}

            def pv_step(p):
                j, t = divmod(p, NKB)
                if t == 0:
                    pv_tiles[j] = (
                        psV.tile([DH + 1, SQ], F32, tag="psV", name="pva"),
                        psV.tile([DH + 1, SQ], F32, tag="psV", name="pvb"))
                pva, pvb = pv_tiles[j]
                pta, ptb = pts.pop(p)
                st, sp = (t == 0), (t == NKB - 1)
                for qc in range(2):
                    s = qc * 512
                    mm(pva[:, s:s + 512], vaug[t][:, 2 * j, :],
                       pta[:, s:s + 512], start=st, stop=sp)
                    mm(pvb[:, s:s + 512], vaug[t][:, 2 * j + 1, :],
                       ptb[:, s:s + 512], start=st, stop=sp)

            # ---- pair finalize (pairs 0..2) ------------------------------
            def part2_copy(j):
                # release the psV tiles right away via DVE copies
                pva, pvb = pv_tiles[j]
                pvsa = cst.tile([DH + 1, SQ], F32, tag="pvsa", bufs=2,
                                name="pvsa")
                pvsb = cst.tile([DH + 1, SQ], F32, tag="pvsb", bufs=2,
                                name="pvsb")
                for qc in range(2):
                    s = qc * 512
                    nc.vector.tensor_copy(out=pvsa[:, s:s + 512],
                                          in_=pva[:, s:s + 512])
                    nc.vector.tensor_copy(out=pvsb[:, s:s + 512],
                                          in_=pvb[:, s:s + 512])
                pv_tiles[j] = (pvsa, pvsb)

            def part2_recip(j):
                # 1/x = exp(-ln(x)) on ACT, reading the rowsum rows directly
                pvsa, pvsb = pv_tiles[j]
                lg = lg_t[j % 2]
                nc.scalar.activation(lg[0:1, :], pvsa[DH:DH + 1, :], LOGF)
                nc.scalar.activation(lg[32:33, :], pvsb[DH:DH + 1, :], LOGF)
                recr = recr_t[j % 2]
                nc.scalar.activation(recr[:], lg[:], EXPF, scale=-1.0)
                return recr

            recrs = {}

            def part2_norm(j):
                pvsa, pvsb = pv_tiles.pop(j)
                recr = recrs.pop(j)
                for qc in range(2):
                    s = qc * 512
                    rbp = psS.tile([P, SQ], F32, tag="psS", name="rbp")
                    mm(rbp[:, 0:512], sel2[:], recr[:, s:s + 512],
                       start=True, stop=True)
                    nc.vector.tensor_tensor(headt[j][0:DH, s:s + 512],
                                            pvsa[0:DH, s:s + 512],
                                            rbp[0:DH, 0:512], MULT)
                    nc.vector.tensor_tensor(headt[j][DH:P, s:s + 512],
                                            pvsb[0:DH, s:s + 512],
                                            rbp[DH:P, 0:512], MULT)

            # ---- static extras schedule (chunk pairs per position) -------
            extras = {}

            def at(pos, f):
                extras.setdefault(pos, []).append(f)

            at(1, lambda: kproj_c(0, 1))
            at(1, lambda: qproj_h(1, 0))
            at(3, lambda: kproj_c(1, 0))
            at(3, lambda: qproj_h(1, 1))
            at(5, lambda: kproj_c(0, 2))
            at(5, lambda: kproj_c(0, 3))
            at(7, lambda: kproj_c(1, 1))
            at(7, lambda: kproj_c(1, 2))
            at(9, lambda: kproj_c(1, 3))
            at(9, lambda: qproj_h(2, 0))
            # late-deadline chunks go into the otherwise-light mid region
            at(20, lambda: qproj_h(2, 1))
            at(20, lambda: kproj_c(2, 0))
            at(24, lambda: kproj_c(2, 1))
            at(24, lambda: kproj_c(2, 2))
            at(18, lambda: kproj_c(2, 3))
            at(18, lambda: qproj_h(3, 0))
            at(28, lambda: qproj_h(3, 1))
            at(28, lambda: kproj_c(3, 0))
            at(34, lambda: kproj_c(3, 1))
            at(34, lambda: kproj_c(3, 2))
            at(38, lambda: kproj_c_sub(3, 3))
            for i in range(NKB // 2):
                at(2 * i + 2, lambda i=i: vproj(2 * i))
                at(2 * i + 2, lambda i=i: vproj(2 * i + 1))
            for qb in range(NQB):
                at(55 + qb // 2, lambda qb=qb: outproj_partial(qb))

            heavy = {p for p, fs in extras.items() if len(fs) >= 2}

            # dummy matmul pairs: keep the PE oversubscribed (p-state stays
            # at full clock) and sacrificially absorb the psS ring's
            # wait-on-exp so real score matmuls never carry it.
            def dummy_fill(nmm):
                done = 0.0
                while done < nmm:
                    for _ in range(2):
                        d = psS.tile([P, SQ], F32, tag="psS", name="dmy")
                        mm(d[:, 0:512], warm[:, 0:P], warm[:],
                           start=True, stop=True)
                        done += 1

            # ---- prologue: warm the PE clock while the first DMAs land ----
            def warmup(n):
                for w in range(n):
                    wps = psS.tile([P, SQ], F32, tag="psS", name="wps")
                    for r in range(3):
                        mm(wps[:, (r % 2) * 512:(r % 2) * 512 + 512],
                           warm[:, 0:P], warm[:], start=True, stop=True)

            warmup(4)
            qproj_h_sub(0, 0)
            kproj_c_sub(0, 0)
            warmup(1)
            qproj_h(0, 1)

            # ---- main stream ---------------------------------------------
            LAGMIN = 3
            SS = NJ * NKB
            pvq = deque()
            deferred = {}

            def defer(pos, f):
                deferred.setdefault(pos, []).append(f)

            def drain_pv(pos, want):
                done = 0
                while done < want and pvq and pvq[0] + LAGMIN <= pos:
                    p = pvq.popleft()
                    pv_step(p)
                    done += 1
                    pj, ptk = divmod(p, NKB)
                    if ptk == NKB - 1:
                        if pj < NJ - 1:
                            part2_copy(pj)
                            defer(pos + 1, lambda pj=pj: recrs.__setitem__(
                                pj, part2_recip(pj)))
                            defer(pos + 2, lambda pj=pj: part2_norm(pj))
                        # let the copies land before the next pair's psV use
                        break
                return done

            for pos in range(SS):
                mmcnt[0] = 0.0
                scores_step(pos)
                pvq.append(pos)
                # early region: hold back pv to build the smoothing backlog;
                # later, always drain first so extras chunks never inherit
                # psa's ring slot (they would stall on this pos's own exp)
                if not (pos in heavy and pos < 17 and len(pvq) < 7):
                    drain_pv(pos, 1)
                for f in extras.get(pos, []):
                    f()
                tries = 0
                while mmcnt[0] < 11 and tries < 2:
                    if not drain_pv(pos, 1):
                        break
                    tries += 1
                # deferred pair-finalize last: its rbp psum allocations then
                # land late enough that the ring's wait-on-exp is pre-resolved
                for f in deferred.pop(pos, []):
                    f()
                if mmcnt[0] < 12:
                    dummy_fill(12 - mmcnt[0])

            # ---- drain remaining pv + deferred finalize ------------------
            pos = SS
            while pvq:
                for f in deferred.pop(pos, []):
                    f()
                drain_pv(pos, 2)
                pos += 1
            for p_ in sorted(deferred):
                for f in deferred[p_]:
                    f()
            deferred.clear()

            # ---- tail: pair 3 per query-half, output add fused on PE -----
            # Normalize and output-project in 256-column pieces so the first
            # output blocks start as soon as possible; dummy matmuls keep the
            # PE clock up across the serial ACT/DVE dependency chain.
            jl = NJ - 1
            pva, pvb = pv_tiles.pop(jl)
            for qh in range(2):
                s = qh * 512
                nc.scalar.activation(lg3[0:1, s:s + 512],
                                     pva[DH:DH + 1, s:s + 512], LOGF)
                nc.scalar.activation(lg3[32:33, s:s + 512],
                                     pvb[DH:DH + 1, s:s + 512], LOGF)
                nc.scalar.activation(recr3[:, s:s + 512], lg3[:, s:s + 512],
                                     EXPF, scale=-1.0)
                rbp = psS.tile([P, SQ], F32, tag="psS", name="rbp3")
                mm(rbp[:, 0:512], sel2[:], recr3[:, s:s + 512],
                   start=True, stop=True)
                dummy_fill(4)
                for h in range(2):
                    c = s + h * 256
                    nc.vector.tensor_copy(out=rbr3[:, c:c + 256],
                                          in_=rbp[:, h * 256:h * 256 + 256])
                    nc.vector.tensor_tensor(headt[jl][0:DH, c:c + 256],
                                            pva[0:DH, c:c + 256],
                                            rbr3[0:DH, c:c + 256], MULT)
                    nc.vector.tensor_tensor(headt[jl][DH:P, c:c + 256],
                                            pvb[0:DH, c:c + 256],
                                            rbr3[DH:P, c:c + 256], MULT)
                    for qb in range(qh * 4 + 2 * h, qh * 4 + 2 * h + 2):
                        ps = psS.tile([P, SQ], F32, tag="psS", name="pst")
                        mm(ps[:, 0:512],
                           headt[jl][:, qb * P:(qb + 1) * P],
                           wm[:, jl, :], start=True, stop=False)
                        mm(ps[:, 0:512], ident[:], pout[qb][:],
                           start=False, stop=True)
                        ot = cst.tile([P, D], F32, tag=f"ot{qb % 2}", bufs=2,
                                      name="ot")
                        if qb % 2 == 0:
                            nc.vector.tensor_copy(out=ot[:], in_=ps[:, 0:512])
                            nc.sync.dma_start(out_d[qb * P:(qb + 1) * P, :],
                                              ot[:])
                        else:
                            nc.scalar.copy(ot[:], ps[:, 0:512])
                            nc.gpsimd.dma_start(out_d[qb * P:(qb + 1) * P, :],
                                                ot[:])

    _split_waits(nc)
    return nc


_NC = None


def _get_nc():
    global _NC
    if _NC is None:
        _NC = _build_nc()
    return _NC


def _prep_in_maps(Q, K, V, W_Q, W_K, W_V, W_gen_S, W_multi_head):
    bf = ml_dtypes.bfloat16
    wq = np.ascontiguousarray(np.asarray(W_Q, np.float32)).astype(bf)
    wv = np.ascontiguousarray(np.asarray(W_V, np.float32)).astype(bf)
    wm = np.ascontiguousarray(np.asarray(W_multi_head, np.float32)).astype(bf)
    # fold W_gen_S into W_K: K_hw = K @ W_K @ blockdiag(W_gen_S)
    wk_f = np.asarray(W_K, np.float32)
    wg_f = np.asarray(W_gen_S, np.float32)
    wkg = np.einsum('dhe,ef->dhf', wk_f.reshape(D, H, DH), wg_f)
    wkg = np.ascontiguousarray(wkg.reshape(D, D)).astype(bf)
    ident = np.eye(P, dtype=np.float32).astype(bf)

    Q = np.asarray(Q, np.float32)
    K = np.asarray(K, np.float32)
    V = np.asarray(V, np.float32)

    in_maps = []
    for c in range(8):
        b, qh = divmod(c, 2)
        qt = np.ascontiguousarray(
            Q[b, qh * SQ:(qh + 1) * SQ, :].T).astype(bf)
        kt = np.ascontiguousarray(K[b].T).astype(bf)
        vt = np.ascontiguousarray(V[b].T).astype(bf)
        in_maps.append({"qt": qt, "kt": kt, "vt": vt, "wq": wq, "wkg": wkg,
                        "wv": wv, "wm": wm, "ident": ident})
    return in_maps


def _run(in_maps, trace=False):
    nc = _get_nc()
    res = run_bass_kernel_spmd(nc, in_maps, list(range(8)), trace=trace)
    out = np.empty((4, SK, D), np.float32)
    for c in range(8):
        b, qh = divmod(c, 2)
        out[b, qh * SQ:(qh + 1) * SQ, :] = res.results[c]["out"]
    return out, res


def kernel(Q, K, V, M, W_Q, W_K, W_V, W_gen_S, W_multi_head):
    in_maps = _prep_in_maps(Q, K, V, W_Q, W_K, W_V, W_gen_S, W_multi_head)
    out, _ = _run(in_maps, trace=False)
    return out


def kernel_traced(Q, K, V, M, W_Q, W_K, W_V, W_gen_S, W_multi_head):
    in_maps = _prep_in_maps(Q, K, V, W_Q, W_K, W_V, W_gen_S, W_multi_head)
    return _run(in_maps, trace=True)
